# revision 19
# baseline (speedup 1.0000x reference)
"""Trainium2 Bass kernel for nn_Gate_Net (Toeplitz + hard-sigmoid prob + cumprod gate).

Reference (per document row of 1024 scores):
  s = doc[1:-1]                                  # n = 1022
  hat[m, j] = s[j-1-m] if j-1-m >= 0 else 0      # [1021, 1022]
  p[m, j]  = clamp(10*(hat - s[j]) + 1, 0, 1)    # hard branch, res = 0.1
  fwd = cumprod(p, axis=0); bwd = same with s reversed
  out = stack([fwd, bwd]) per doc -> full [32, 2, 1021, 1022] f32

Key structure: with v = 10*s and c_j = 1 - v_j, factor(j, m) =
clamp(v[j-1-m] + c_j, 0, 1) (v[<0] := 0 reproduces the boundary rule).
A column's cumprod hits EXACT 0 at the first m with v[j-1-m] + c_j <= 0,
and everything below stays 0.  On real inputs ~99% of columns die within
the first K=128 rows, so:

  1. Prefix pass (device): rows 0..K-1 for all (padded) 1024 columns of
     all 8 doc-dirs at once.  Partition p = (dd, col-block-of-64); free
     axis t = j'*K + m.  q built from a shifted AP over a per-partition
     slice of v plus a broadcast c, clamped, then ONE segmented
     tensor_tensor_scan (scan: state = data0*state + data1; at each
     column start data0=0/data1=q0 resets the chain).  Result is DMAd
     with 128 contiguous 32 KiB descriptors -- no transpose needed; the
     host reorders (col-major -> row-major) on 4 MiB/core.
  2. Survivor pass (device): columns with no exact-zero factor among
     rows < K (found host-side with a sliding-window min; ~130/core)
     are scanned at full length col-major and the host scatters
     rows K.. into the output.
  3. Everything else is exactly 0 and is never written (host assembles
     into np.zeros).

Sharding: pure data parallel, 4 docs (8 doc-dirs) per core.
"""
import numpy as np

import concourse.bass as bass
import concourse.bacc as bacc
import concourse.tile as tile
from concourse import mybir
from concourse import bass_utils

P = 128            # SBUF partitions
L = 1024           # sentences per document
N = L - 2          # 1022 real columns per doc-dir
ROWS = N - 1       # 1021 output rows
K = 48             # dense prefix rows computed for every column
NCOL = 1024        # padded column count (cols N..NCOL-1 are garbage)
CJ = NCOL // 16    # 64 columns per partition slot
FREE = CJ * K      # 8192 free elems per partition in the prefix pass
ARRW = 2560        # [K zeros][1022 v][pad] at 0..1280, [1024 c][pad] at 1280..
COFF = 1280        # offset of the c region inside an arr row
SURV_ROWS = ROWS - K   # rows written per survivor column

_NC_CACHE: dict = {}


def _ap(t: bass.AP, delta: int, dims):
    """Custom free-dim AP over tile t (keeps t's partition pair)."""
    return bass.AP(tensor=t.tensor, offset=t.offset + delta,
                   ap=[list(t.ap[0])] + [list(d) for d in dims])


def build_nc(n_dd: int, surv_tiles: tuple):
    """Bass program: prefix pass for n_dd=8 doc-dirs + survivor scans."""
    assert n_dd == 8
    nc = bacc.Bacc("TRN2", target_bir_lowering=False, debug=False, num_devices=8)
    arr = nc.dram_tensor("arr", [n_dd, ARRW], mybir.dt.float32, kind="ExternalInput")
    cap = sum(surv_tiles)
    if cap:
        sc = nc.dram_tensor("sc", [cap, L], mybir.dt.float32, kind="ExternalInput")
        s1 = nc.dram_tensor("s1", [cap, SURV_ROWS], mybir.dt.float32,
                            kind="ExternalOutput")
    s0 = nc.dram_tensor("s0", [P, FREE], mybir.dt.float32, kind="ExternalOutput")

    add = mybir.AluOpType.add
    mult = mybir.AluOpType.mult
    amin = mybir.AluOpType.min
    amax = mybir.AluOpType.max

    with tile.TileContext(nc) as tc:
        with (
            tc.tile_pool(name="io", bufs=1) as io,
            tc.tile_pool(name="work", bufs=1) as work,
        ):
            # ---- prefix pass -------------------------------------------------
            # arr_sb[p, t] = v[J0 + t - K], J0 = (p % 16) * 64   (p = dd*16 + slot)
            arr_sb = io.tile([P, K + CJ], mybir.dt.float32)
            nc.sync.dma_start(
                out=arr_sb[:],
                in_=bass.AP(tensor=arr, offset=0,
                            ap=[[ARRW, 8], [CJ, 16], [1, K + CJ]]),
            )
            # c[p, j'] = 1 - v[J0 + j']  (host-precomputed, own region of arr).
            # Loaded on the Activation HWDGE ring, parallel to arr on sync's.
            c_sb = io.tile([P, CJ], mybir.dt.float32)
            nc.scalar.dma_start(
                out=c_sb[:],
                in_=bass.AP(tensor=arr, offset=COFF,
                            ap=[[ARRW, 8], [CJ, 16], [1, CJ]]),
            )
            # survivor inputs early on the sync queue
            zeros = None
            sbs = []
            if cap:
                zeros = io.tile([P, ROWS], mybir.dt.float32)
                off = 0
                for ti, sz in enumerate(surv_tiles):
                    sb = work.tile([P, L], mybir.dt.float32, name=f"sb{ti}")
                    nc.sync.dma_start(out=sb[:sz, :], in_=sc[off:off + sz, :])
                    sbs.append(sb)
                    off += sz

            q = work.tile([P, FREE], mybir.dt.float32)
            qc = work.tile([P, FREE], mybir.dt.float32)
            d1 = work.tile([P, FREE], mybir.dt.float32)
            R = work.tile([P, FREE], mybir.dt.float32)
            nchunk = 4
            csz = FREE // nchunk
            JV = CJ - CJ // nchunk          # j'-slots computed on vector
            # q[p, j'*K + m] = v[J0 + j' - 1 - m] + c[J0 + j'] for m >= 1;
            # m == 0 slots of q are pre-zeroed (the scan's segment reset reads
            # data0 = 0 there) and the m == 0 factor value goes into d1.
            def q_build(eng, j0, j1):
                n = j1 - j0
                eng.tensor_tensor(
                    out=_ap(q, j0 * K + 1, [[K, n], [1, K - 1]]),
                    in0=_ap(arr_sb, K - 2 + j0, [[1, n], [-1, K - 1]]),
                    in1=_ap(c_sb, j0, [[1, n], [0, K - 1]]),
                    op=add,
                )
                eng.tensor_scalar(
                    out=qc[:, j0 * K:j1 * K], in0=q[:, j0 * K:j1 * K],
                    scalar1=1.0, scalar2=0.0, op0=amin, op1=amax,
                )
                # d1 m0 value = clamp(v[J0+j'-1] + c[J0+j'])
                eng.tensor_tensor(
                    out=_ap(d1, j0 * K, [[K, n]]),
                    in0=_ap(arr_sb, K - 1 + j0, [[1, n]]),
                    in1=_ap(c_sb, j0, [[1, n]]),
                    op=add,
                )
                eng.tensor_scalar(
                    out=_ap(d1, j0 * K, [[K, n]]), in0=_ap(d1, j0 * K, [[K, n]]),
                    scalar1=1.0, scalar2=0.0, op0=amin, op1=amax,
                )

            # gpsimd: early zero of q's m0 slots + d1 + zeros, then its q share
            nc.gpsimd.memset(_ap(q, 0, [[K, CJ]]), 0.0)
            nc.gpsimd.memset(d1[:], 0.0)
            if cap:
                nc.gpsimd.memset(zeros[:], 0.0)
                sz0 = surv_tiles[0]
                nc.gpsimd.tensor_scalar(
                    out=sbs[0][:sz0, 0:ROWS], in0=sbs[0][:sz0, 0:ROWS],
                    scalar1=1.0, scalar2=0.0, op0=amin, op1=amax,
                )
            q_build(nc.gpsimd, JV, CJ)
            if cap:
                off = surv_tiles[0]
                for ti, sz in list(enumerate(surv_tiles))[1:]:
                    nc.gpsimd.tensor_scalar(
                        out=sbs[ti][:sz, 0:ROWS], in0=sbs[ti][:sz, 0:ROWS],
                        scalar1=1.0, scalar2=0.0, op0=amin, op1=amax,
                    )

            # vector: its q share, then scans with survivor scans interleaved
            q_build(nc.vector, 0, JV)

            def svscan(ti, off):
                sz = surv_tiles[ti]
                rs = work.tile([P, ROWS], mybir.dt.float32, name=f"rs{ti}")
                nc.vector.tensor_tensor_scan(
                    out=rs[:sz, :], data0=sbs[ti][:sz, 0:ROWS],
                    data1=zeros[:sz, :], initial=1.0, op0=mult, op1=add,
                )
                nc.sync.dma_start(out=s1[off:off + sz, :], in_=rs[:sz, K:ROWS])

            for ch in range(nchunk - 1):
                sl = slice(ch * csz, (ch + 1) * csz)
                nc.vector.tensor_tensor_scan(
                    out=R[:, sl], data0=qc[:, sl], data1=d1[:, sl],
                    initial=0.0, op0=mult, op1=add,
                )
                nc.sync.dma_start(out=s0[:, sl], in_=R[:, sl])
            if cap:
                svscan(0, 0)
            sl = slice((nchunk - 1) * csz, FREE)
            nc.vector.tensor_tensor_scan(
                out=R[:, sl], data0=qc[:, sl], data1=d1[:, sl],
                initial=0.0, op0=mult, op1=add,
            )
            nc.sync.dma_start(out=s0[:, sl], in_=R[:, sl])
            if cap:
                off = surv_tiles[0]
                for ti, sz in list(enumerate(surv_tiles))[1:]:
                    svscan(ti, off)
                    off += sz
    nc.compile()
    return nc


def get_nc(n_dd: int, surv_tiles: tuple):
    key = (n_dd, surv_tiles)
    if key not in _NC_CACHE:
        _NC_CACHE[key] = build_nc(n_dd, surv_tiles)
    return _NC_CACHE[key]


def _find_survivors(v: np.ndarray):
    """v: [1022] f32 (10*s).  Return j-indices with no exact-zero factor in
    rows m < K.  Factor zero <=> f32(v[j-1-m] + c_j) <= 0 (c = 1 - v), or,
    for the boundary rows (j <= m < K), c_j <= 0."""
    n = v.shape[0]
    c = (np.float32(1.0) - v).astype(np.float32)
    m = np.full(n, np.inf, dtype=np.float32)          # min of v over window
    if n > K:
        w = np.lib.stride_tricks.sliding_window_view(v, K).min(axis=1)
        m[K:] = w[:-1]                                # j >= K: v[j-K:j]
    run = np.minimum.accumulate(v)
    m[1:K] = run[:K - 1]                              # 0 < j < K: v[0:j]
    dead = (m + c).astype(np.float32) <= 0.0
    jk = np.arange(n) < K
    dead |= jk & (c <= 0.0)
    return np.nonzero(~dead)[0]


def prepare(score: np.ndarray, score_idx: np.ndarray):
    """Build (nc, in_maps, assemble) for the given inputs.  assemble(results)
    turns the per-core result dicts into the full output array."""
    score = np.asarray(score, dtype=np.float32)
    score_idx = np.asarray(score_idx)
    docs = score[score_idx]                  # [B, L]
    Bn, Ln = docs.shape
    assert Ln == L
    n_cores = 8
    dpc = Bn // n_cores                      # docs per core
    n_dd = dpc * 2
    assert n_dd == 8

    # per-core v arrays and survivor lists
    vs = []                                  # vs[core][dd] = v (f32 [1022])
    survs = []                               # survs[core] = list[(dd, j)]
    for cid in range(n_cores):
        vcore, scount = [], []
        for dl in range(dpc):
            s = docs[cid * dpc + dl, 1:-1].astype(np.float32)
            for t in range(2):
                sd = s if t == 0 else s[::-1]
                vcore.append((np.float32(10.0) * sd).astype(np.float32))
        slist = []
        for dd in range(n_dd):
            for j in _find_survivors(vcore[dd]):
                slist.append((dd, int(j)))
        vs.append(vcore)
        survs.append(slist)

    max_surv = max(len(s) for s in survs)
    tiles = []
    rem = max_surv
    while rem > 0:
        t = min(P, rem)
        if t < P:
            t = max(32, -(-t // 32) * 32)
        tiles.append(t)
        rem -= t
    surv_tiles = tuple(tiles)
    cap = sum(surv_tiles)

    in_maps = []
    for cid in range(n_cores):
        arr = np.zeros((n_dd, ARRW), np.float32)
        for dd in range(n_dd):
            v = vs[cid][dd]
            arr[dd, K:K + N] = v
            arr[dd, COFF:COFF + N] = (np.float32(1.0) - v).astype(np.float32)
        im = {"arr": arr}
        if cap:
            scm = np.zeros((cap, L), np.float32)
            for slot, (dd, j) in enumerate(survs[cid]):
                v = vs[cid][dd]
                cj = np.float32(1.0) - v[j]
                hat = np.zeros(ROWS, np.float32)
                if j > 0:
                    hat[:j] = v[j - 1::-1]
                scm[slot, :ROWS] = (hat + cj).astype(np.float32)
            im["sc"] = scm
        in_maps.append(im)

    nc = get_nc(n_dd, surv_tiles)

    def assemble(results):
        full = np.zeros((Bn, 2, ROWS, N), np.float32)
        for cid in range(n_cores):
            r = results[cid]
            # prefix: [128, FREE] -> [dd, slot, j', m] -> [dd, m, col]
            pref = np.asarray(r["s0"]).reshape(n_dd, 16, CJ, K)
            pref = pref.transpose(0, 3, 1, 2).reshape(n_dd, K, NCOL)[:, :, :N]
            for dd in range(n_dd):
                doc, t = cid * dpc + dd // 2, dd % 2
                full[doc, t, :K, :] = pref[dd]
            if cap:
                s1v = np.asarray(r["s1"])
                for slot, (dd, j) in enumerate(survs[cid]):
                    doc, t = cid * dpc + dd // 2, dd % 2
                    full[doc, t, K:, j] = s1v[slot]
        return full

    return nc, in_maps, assemble


def kernel(score: np.ndarray, score_idx: np.ndarray) -> np.ndarray:
    nc, in_maps, assemble = prepare(score, score_idx)
    res = bass_utils.run_bass_kernel_spmd(nc, in_maps, core_ids=list(range(8)))
    return assemble(res.results)


# revision 20
# speedup vs baseline: 1.0286x; 1.0286x over previous
"""Trainium2 Bass kernel for nn_Gate_Net (Toeplitz + hard-sigmoid prob + cumprod gate).

Reference (per document row of 1024 scores):
  s = doc[1:-1]                                  # n = 1022
  hat[m, j] = s[j-1-m] if j-1-m >= 0 else 0      # [1021, 1022]
  p[m, j]  = clamp(10*(hat - s[j]) + 1, 0, 1)    # hard branch, res = 0.1
  fwd = cumprod(p, axis=0); bwd = same with s reversed
  out = stack([fwd, bwd]) per doc -> full [32, 2, 1021, 1022] f32

Key structure: with v = 10*s and c_j = 1 - v_j, factor(j, m) =
clamp(v[j-1-m] + c_j, 0, 1) (v[<0] := 0 reproduces the boundary rule).
A column's cumprod hits EXACT 0 at the first m with v[j-1-m] + c_j <= 0,
and everything below stays 0.  On real inputs ~99% of columns die within
the first K=128 rows, so:

  1. Prefix pass (device): rows 0..K-1 for all (padded) 1024 columns of
     all 8 doc-dirs at once.  Partition p = (dd, col-block-of-64); free
     axis t = j'*K + m.  q built from a shifted AP over a per-partition
     slice of v plus a broadcast c, clamped, then ONE segmented
     tensor_tensor_scan (scan: state = data0*state + data1; at each
     column start data0=0/data1=q0 resets the chain).  Result is DMAd
     with 128 contiguous 32 KiB descriptors -- no transpose needed; the
     host reorders (col-major -> row-major) on 4 MiB/core.
  2. Survivor pass (device): columns with no exact-zero factor among
     rows < K (found host-side with a sliding-window min; ~130/core)
     are scanned at full length col-major and the host scatters
     rows K.. into the output.
  3. Everything else is exactly 0 and is never written (host assembles
     into np.zeros).

Sharding: pure data parallel, 4 docs (8 doc-dirs) per core.
"""
import numpy as np

import concourse.bass as bass
import concourse.bacc as bacc
import concourse.tile as tile
from concourse import mybir
from concourse import bass_utils

P = 128            # SBUF partitions
L = 1024           # sentences per document
N = L - 2          # 1022 real columns per doc-dir
ROWS = N - 1       # 1021 output rows
K = 32             # dense prefix rows computed for every column
NCOL = 1024        # padded column count (cols N..NCOL-1 are garbage)
CJ = NCOL // 16    # 64 columns per partition slot
FREE = CJ * K      # 8192 free elems per partition in the prefix pass
ARRW = 2560        # [K zeros][1022 v][pad] at 0..1280, [1024 c][pad] at 1280..
COFF = 1280        # offset of the c region inside an arr row
SURV_ROWS = ROWS - K   # rows written per survivor column

_NC_CACHE: dict = {}


def _ap(t: bass.AP, delta: int, dims):
    """Custom free-dim AP over tile t (keeps t's partition pair)."""
    return bass.AP(tensor=t.tensor, offset=t.offset + delta,
                   ap=[list(t.ap[0])] + [list(d) for d in dims])


def build_nc(n_dd: int, surv_tiles: tuple):
    """Bass program: prefix pass for n_dd=8 doc-dirs + survivor scans."""
    assert n_dd == 8
    nc = bacc.Bacc("TRN2", target_bir_lowering=False, debug=False, num_devices=8)
    arr = nc.dram_tensor("arr", [n_dd, ARRW], mybir.dt.float32, kind="ExternalInput")
    cap = sum(surv_tiles)
    if cap:
        sc = nc.dram_tensor("sc", [cap, L], mybir.dt.float32, kind="ExternalInput")
        s1 = nc.dram_tensor("s1", [cap, SURV_ROWS], mybir.dt.float32,
                            kind="ExternalOutput")
    s0 = nc.dram_tensor("s0", [P, FREE], mybir.dt.float32, kind="ExternalOutput")

    add = mybir.AluOpType.add
    mult = mybir.AluOpType.mult
    amin = mybir.AluOpType.min
    amax = mybir.AluOpType.max

    with tile.TileContext(nc) as tc:
        with (
            tc.tile_pool(name="io", bufs=1) as io,
            tc.tile_pool(name="work", bufs=1) as work,
        ):
            # ---- prefix pass -------------------------------------------------
            # arr_sb[p, t] = v[J0 + t - K], J0 = (p % 16) * 64   (p = dd*16 + slot)
            arr_sb = io.tile([P, K + CJ], mybir.dt.float32)
            nc.sync.dma_start(
                out=arr_sb[:],
                in_=bass.AP(tensor=arr, offset=0,
                            ap=[[ARRW, 8], [CJ, 16], [1, K + CJ]]),
            )
            # c[p, j'] = 1 - v[J0 + j']  (host-precomputed, own region of arr).
            # Loaded on the Activation HWDGE ring, parallel to arr on sync's.
            c_sb = io.tile([P, CJ], mybir.dt.float32)
            nc.scalar.dma_start(
                out=c_sb[:],
                in_=bass.AP(tensor=arr, offset=COFF,
                            ap=[[ARRW, 8], [CJ, 16], [1, CJ]]),
            )
            # survivor inputs early on the sync queue
            zeros = None
            sbs = []
            if cap:
                zeros = io.tile([P, ROWS], mybir.dt.float32)
                off = 0
                for ti, sz in enumerate(surv_tiles):
                    sb = work.tile([P, L], mybir.dt.float32, name=f"sb{ti}")
                    nc.sync.dma_start(out=sb[:sz, :], in_=sc[off:off + sz, :])
                    sbs.append(sb)
                    off += sz

            q = work.tile([P, FREE], mybir.dt.float32)
            qc = work.tile([P, FREE], mybir.dt.float32)
            d1 = work.tile([P, FREE], mybir.dt.float32)
            R = work.tile([P, FREE], mybir.dt.float32)
            nchunk = 4
            csz = FREE // nchunk
            JV = CJ - CJ // nchunk          # j'-slots computed on vector
            # q[p, j'*K + m] = v[J0 + j' - 1 - m] + c[J0 + j'] for m >= 1;
            # m == 0 slots of q are pre-zeroed (the scan's segment reset reads
            # data0 = 0 there) and the m == 0 factor value goes into d1.
            def q_build(eng, j0, j1):
                n = j1 - j0
                eng.tensor_tensor(
                    out=_ap(q, j0 * K + 1, [[K, n], [1, K - 1]]),
                    in0=_ap(arr_sb, K - 2 + j0, [[1, n], [-1, K - 1]]),
                    in1=_ap(c_sb, j0, [[1, n], [0, K - 1]]),
                    op=add,
                )
                eng.tensor_scalar(
                    out=qc[:, j0 * K:j1 * K], in0=q[:, j0 * K:j1 * K],
                    scalar1=1.0, scalar2=0.0, op0=amin, op1=amax,
                )
                # d1 m0 value = clamp(v[J0+j'-1] + c[J0+j'])
                eng.tensor_tensor(
                    out=_ap(d1, j0 * K, [[K, n]]),
                    in0=_ap(arr_sb, K - 1 + j0, [[1, n]]),
                    in1=_ap(c_sb, j0, [[1, n]]),
                    op=add,
                )
                eng.tensor_scalar(
                    out=_ap(d1, j0 * K, [[K, n]]), in0=_ap(d1, j0 * K, [[K, n]]),
                    scalar1=1.0, scalar2=0.0, op0=amin, op1=amax,
                )

            # gpsimd: early zero of q's m0 slots + d1 + zeros, then its q share
            nc.gpsimd.memset(_ap(q, 0, [[K, CJ]]), 0.0)
            nc.gpsimd.memset(d1[:], 0.0)
            if cap:
                nc.gpsimd.memset(zeros[:], 0.0)
                sz0 = surv_tiles[0]
                nc.gpsimd.tensor_scalar(
                    out=sbs[0][:sz0, 0:ROWS], in0=sbs[0][:sz0, 0:ROWS],
                    scalar1=1.0, scalar2=0.0, op0=amin, op1=amax,
                )
            q_build(nc.gpsimd, JV, CJ)
            if cap:
                off = surv_tiles[0]
                for ti, sz in list(enumerate(surv_tiles))[1:]:
                    nc.gpsimd.tensor_scalar(
                        out=sbs[ti][:sz, 0:ROWS], in0=sbs[ti][:sz, 0:ROWS],
                        scalar1=1.0, scalar2=0.0, op0=amin, op1=amax,
                    )

            # vector: its q share, then scans with survivor scans interleaved
            q_build(nc.vector, 0, JV)

            def svscan(ti, off):
                sz = surv_tiles[ti]
                rs = work.tile([P, ROWS], mybir.dt.float32, name=f"rs{ti}")
                nc.vector.tensor_tensor_scan(
                    out=rs[:sz, :], data0=sbs[ti][:sz, 0:ROWS],
                    data1=zeros[:sz, :], initial=1.0, op0=mult, op1=add,
                )
                nc.sync.dma_start(out=s1[off:off + sz, :], in_=rs[:sz, K:ROWS])

            for ch in range(nchunk - 1):
                sl = slice(ch * csz, (ch + 1) * csz)
                nc.vector.tensor_tensor_scan(
                    out=R[:, sl], data0=qc[:, sl], data1=d1[:, sl],
                    initial=0.0, op0=mult, op1=add,
                )
                nc.sync.dma_start(out=s0[:, sl], in_=R[:, sl])
            if cap:
                svscan(0, 0)
            sl = slice((nchunk - 1) * csz, FREE)
            nc.vector.tensor_tensor_scan(
                out=R[:, sl], data0=qc[:, sl], data1=d1[:, sl],
                initial=0.0, op0=mult, op1=add,
            )
            nc.sync.dma_start(out=s0[:, sl], in_=R[:, sl])
            if cap:
                off = surv_tiles[0]
                for ti, sz in list(enumerate(surv_tiles))[1:]:
                    svscan(ti, off)
                    off += sz
    nc.compile()
    return nc


def get_nc(n_dd: int, surv_tiles: tuple):
    key = (n_dd, surv_tiles)
    if key not in _NC_CACHE:
        _NC_CACHE[key] = build_nc(n_dd, surv_tiles)
    return _NC_CACHE[key]


def _find_survivors(v: np.ndarray):
    """v: [1022] f32 (10*s).  Return j-indices with no exact-zero factor in
    rows m < K.  Factor zero <=> f32(v[j-1-m] + c_j) <= 0 (c = 1 - v), or,
    for the boundary rows (j <= m < K), c_j <= 0."""
    n = v.shape[0]
    c = (np.float32(1.0) - v).astype(np.float32)
    m = np.full(n, np.inf, dtype=np.float32)          # min of v over window
    if n > K:
        w = np.lib.stride_tricks.sliding_window_view(v, K).min(axis=1)
        m[K:] = w[:-1]                                # j >= K: v[j-K:j]
    run = np.minimum.accumulate(v)
    m[1:K] = run[:K - 1]                              # 0 < j < K: v[0:j]
    dead = (m + c).astype(np.float32) <= 0.0
    jk = np.arange(n) < K
    dead |= jk & (c <= 0.0)
    return np.nonzero(~dead)[0]


def prepare(score: np.ndarray, score_idx: np.ndarray):
    """Build (nc, in_maps, assemble) for the given inputs.  assemble(results)
    turns the per-core result dicts into the full output array."""
    score = np.asarray(score, dtype=np.float32)
    score_idx = np.asarray(score_idx)
    docs = score[score_idx]                  # [B, L]
    Bn, Ln = docs.shape
    assert Ln == L
    n_cores = 8
    dpc = Bn // n_cores                      # docs per core
    n_dd = dpc * 2
    assert n_dd == 8

    # per-core v arrays and survivor lists
    vs = []                                  # vs[core][dd] = v (f32 [1022])
    survs = []                               # survs[core] = list[(dd, j)]
    for cid in range(n_cores):
        vcore, scount = [], []
        for dl in range(dpc):
            s = docs[cid * dpc + dl, 1:-1].astype(np.float32)
            for t in range(2):
                sd = s if t == 0 else s[::-1]
                vcore.append((np.float32(10.0) * sd).astype(np.float32))
        slist = []
        for dd in range(n_dd):
            for j in _find_survivors(vcore[dd]):
                slist.append((dd, int(j)))
        vs.append(vcore)
        survs.append(slist)

    max_surv = max(len(s) for s in survs)
    tiles = []
    rem = max_surv
    while rem > 0:
        t = min(P, rem)
        if t < P:
            t = max(32, -(-t // 32) * 32)
        tiles.append(t)
        rem -= t
    surv_tiles = tuple(tiles)
    cap = sum(surv_tiles)

    in_maps = []
    for cid in range(n_cores):
        arr = np.zeros((n_dd, ARRW), np.float32)
        for dd in range(n_dd):
            v = vs[cid][dd]
            arr[dd, K:K + N] = v
            arr[dd, COFF:COFF + N] = (np.float32(1.0) - v).astype(np.float32)
        im = {"arr": arr}
        if cap:
            scm = np.zeros((cap, L), np.float32)
            for slot, (dd, j) in enumerate(survs[cid]):
                v = vs[cid][dd]
                cj = np.float32(1.0) - v[j]
                hat = np.zeros(ROWS, np.float32)
                if j > 0:
                    hat[:j] = v[j - 1::-1]
                scm[slot, :ROWS] = (hat + cj).astype(np.float32)
            im["sc"] = scm
        in_maps.append(im)

    nc = get_nc(n_dd, surv_tiles)

    def assemble(results):
        full = np.zeros((Bn, 2, ROWS, N), np.float32)
        for cid in range(n_cores):
            r = results[cid]
            # prefix: [128, FREE] -> [dd, slot, j', m] -> [dd, m, col]
            pref = np.asarray(r["s0"]).reshape(n_dd, 16, CJ, K)
            pref = pref.transpose(0, 3, 1, 2).reshape(n_dd, K, NCOL)[:, :, :N]
            for dd in range(n_dd):
                doc, t = cid * dpc + dd // 2, dd % 2
                full[doc, t, :K, :] = pref[dd]
            if cap:
                s1v = np.asarray(r["s1"])
                for slot, (dd, j) in enumerate(survs[cid]):
                    doc, t = cid * dpc + dd // 2, dd % 2
                    full[doc, t, K:, j] = s1v[slot]
        return full

    return nc, in_maps, assemble


def kernel(score: np.ndarray, score_idx: np.ndarray) -> np.ndarray:
    nc, in_maps, assemble = prepare(score, score_idx)
    res = bass_utils.run_bass_kernel_spmd(nc, in_maps, core_ids=list(range(8)))
    return assemble(res.results)


# revision 27
# speedup vs baseline: 1.2263x; 1.1923x over previous
"""Trainium2 Bass kernel for nn_Gate_Net (Toeplitz + hard-sigmoid prob + cumprod gate).

Reference (per document row of 1024 scores):
  s = doc[1:-1]                                  # n = 1022
  hat[m, j] = s[j-1-m] if j-1-m >= 0 else 0      # [1021, 1022]
  p[m, j]  = clamp(10*(hat - s[j]) + 1, 0, 1)    # hard branch, res = 0.1
  fwd = cumprod(p, axis=0); bwd = same with s reversed
  out = stack([fwd, bwd]) per doc -> full [32, 2, 1021, 1022] f32

Key structure: with v = 10*s and c_j = 1 - v_j, factor(j, m) =
clamp(v[j-1-m] + c_j, 0, 1) (v[<0] := 0 reproduces the boundary rule).
A column's cumprod hits EXACT 0 at the first m with v[j-1-m] + c_j <= 0,
and everything below stays 0.  On real inputs ~99% of columns die within
the first K=128 rows, so:

  1. Prefix pass (device): rows 0..K-1 for all (padded) 1024 columns of
     all 8 doc-dirs at once.  Partition p = (dd, col-block-of-64); free
     axis t = j'*K + m.  q built from a shifted AP over a per-partition
     slice of v plus a broadcast c, clamped, then ONE segmented
     tensor_tensor_scan (scan: state = data0*state + data1; at each
     column start data0=0/data1=q0 resets the chain).  Result is DMAd
     with 128 contiguous 32 KiB descriptors -- no transpose needed; the
     host reorders (col-major -> row-major) on 4 MiB/core.
  2. Survivor pass (device): columns with no exact-zero factor among
     rows < K (found host-side with a sliding-window min; ~130/core)
     are scanned at full length col-major and the host scatters
     rows K.. into the output.
  3. Everything else is exactly 0 and is never written (host assembles
     into np.zeros).

Sharding: pure data parallel, 4 docs (8 doc-dirs) per core.
"""
import numpy as np

import concourse.bass as bass
import concourse.bacc as bacc
import concourse.tile as tile
from concourse import mybir
from concourse import bass_utils

P = 128            # SBUF partitions
L = 1024           # sentences per document
N = L - 2          # 1022 real columns per doc-dir
ROWS = N - 1       # 1021 output rows
K = 32             # dense prefix rows computed for every column
NCOL = 1024        # padded column count (cols N..NCOL-1 are garbage)
CJ = NCOL // 16    # 64 columns per partition slot
FREE = CJ * K      # 8192 free elems per partition in the prefix pass
ARRW = 2560        # [K zeros][1022 v][pad] at 0..1280, [1024 c][pad] at 1280..
COFF = 1280        # offset of the c region inside an arr row
SURV_ROWS = ROWS - K   # rows written per survivor column

_NC_CACHE: dict = {}


def _ap(t: bass.AP, delta: int, dims):
    """Custom free-dim AP over tile t (keeps t's partition pair)."""
    return bass.AP(tensor=t.tensor, offset=t.offset + delta,
                   ap=[list(t.ap[0])] + [list(d) for d in dims])


def build_nc(n_dd: int, surv_tiles: tuple):
    """Bass program: prefix pass for n_dd=8 doc-dirs + survivor scans.
    surv_tiles: tuple of (n_slots, scan_len) pairs, scan_len <= ROWS."""
    assert n_dd == 8
    nc = bacc.Bacc("TRN2", target_bir_lowering=False, debug=False, num_devices=8)
    arr = nc.dram_tensor("arr", [n_dd, ARRW], mybir.dt.float32, kind="ExternalInput")
    cap = sum(sz for sz, _ in surv_tiles)
    if cap:
        sc = nc.dram_tensor("sc", [cap, L], mybir.dt.float32, kind="ExternalInput")
        s1 = nc.dram_tensor("s1", [cap, SURV_ROWS], mybir.dt.float32,
                            kind="ExternalOutput")
    s0 = nc.dram_tensor("s0", [P, FREE], mybir.dt.float32, kind="ExternalOutput")

    add = mybir.AluOpType.add
    mult = mybir.AluOpType.mult
    amin = mybir.AluOpType.min
    amax = mybir.AluOpType.max

    with tile.TileContext(nc) as tc:
        with (
            tc.tile_pool(name="io", bufs=1) as io,
            tc.tile_pool(name="work", bufs=1) as work,
        ):
            # ---- prefix pass -------------------------------------------------
            # arr_sb[p, t] = v[J0 + t - K], J0 = (p % 16) * 64   (p = dd*16 + slot)
            arr_sb = io.tile([P, K + CJ], mybir.dt.float32)
            nc.sync.dma_start(
                out=arr_sb[:],
                in_=bass.AP(tensor=arr, offset=0,
                            ap=[[ARRW, 8], [CJ, 16], [1, K + CJ]]),
            )
            # c[p, j'] = 1 - v[J0 + j']  (host-precomputed, own region of arr).
            # Loaded on the Activation HWDGE ring, parallel to arr on sync's.
            c_sb = io.tile([P, CJ], mybir.dt.float32)
            nc.scalar.dma_start(
                out=c_sb[:],
                in_=bass.AP(tensor=arr, offset=COFF,
                            ap=[[ARRW, 8], [CJ, 16], [1, CJ]]),
            )
            # survivor inputs early on the sync queue
            zeros = None
            sbs = []
            if cap:
                zeros = io.tile([P, ROWS], mybir.dt.float32)
                off = 0
                for ti, (sz, ln) in enumerate(surv_tiles):
                    sb = work.tile([P, L], mybir.dt.float32, name=f"sb{ti}")
                    nc.sync.dma_start(out=sb[:sz, 0:ln], in_=sc[off:off + sz, 0:ln])
                    sbs.append(sb)
                    off += sz

            q = work.tile([P, FREE], mybir.dt.float32)
            qc = work.tile([P, FREE], mybir.dt.float32)
            d1 = work.tile([P, FREE], mybir.dt.float32)
            R = work.tile([P, FREE], mybir.dt.float32)
            nchunk = 4
            csz = FREE // nchunk
            JV = CJ - CJ // nchunk          # j'-slots computed on vector
            # q[p, j'*K + m] = v[J0 + j' - 1 - m] + c[J0 + j'] for m >= 1;
            # m == 0 slots of q are pre-zeroed (the scan's segment reset reads
            # data0 = 0 there) and the m == 0 factor value goes into d1.
            def q_build(eng, j0, j1):
                n = j1 - j0
                eng.tensor_tensor(
                    out=_ap(q, j0 * K + 1, [[K, n], [1, K - 1]]),
                    in0=_ap(arr_sb, K - 2 + j0, [[1, n], [-1, K - 1]]),
                    in1=_ap(c_sb, j0, [[1, n], [0, K - 1]]),
                    op=add,
                )
                eng.tensor_scalar(
                    out=qc[:, j0 * K:j1 * K], in0=q[:, j0 * K:j1 * K],
                    scalar1=1.0, scalar2=0.0, op0=amin, op1=amax,
                )
                # d1 m0 value = clamp(v[J0+j'-1] + c[J0+j'])
                eng.tensor_tensor(
                    out=_ap(d1, j0 * K, [[K, n]]),
                    in0=_ap(arr_sb, K - 1 + j0, [[1, n]]),
                    in1=_ap(c_sb, j0, [[1, n]]),
                    op=add,
                )
                eng.tensor_scalar(
                    out=_ap(d1, j0 * K, [[K, n]]), in0=_ap(d1, j0 * K, [[K, n]]),
                    scalar1=1.0, scalar2=0.0, op0=amin, op1=amax,
                )

            # gpsimd: early zero of q's m0 slots + d1 + zeros, then its q share
            nc.gpsimd.memset(_ap(q, 0, [[K, CJ]]), 0.0)
            nc.gpsimd.memset(d1[:], 0.0)
            if cap:
                nc.gpsimd.memset(zeros[:], 0.0)
                sz0, ln0 = surv_tiles[0]
                nc.gpsimd.tensor_scalar(
                    out=sbs[0][:sz0, 0:ln0], in0=sbs[0][:sz0, 0:ln0],
                    scalar1=1.0, scalar2=0.0, op0=amin, op1=amax,
                )
            q_build(nc.gpsimd, JV, CJ)
            if cap:
                for ti, (sz, ln) in list(enumerate(surv_tiles))[1:]:
                    nc.gpsimd.tensor_scalar(
                        out=sbs[ti][:sz, 0:ln], in0=sbs[ti][:sz, 0:ln],
                        scalar1=1.0, scalar2=0.0, op0=amin, op1=amax,
                    )

            # vector: its q share, then scans with survivor scans interleaved
            q_build(nc.vector, 0, JV)

            def svscan(ti, off):
                sz, ln = surv_tiles[ti]
                rs = work.tile([P, ln], mybir.dt.float32, name=f"rs{ti}")
                nc.vector.tensor_tensor_scan(
                    out=rs[:sz, :], data0=sbs[ti][:sz, 0:ln],
                    data1=zeros[:sz, 0:ln], initial=1.0, op0=mult, op1=add,
                )
                nc.sync.dma_start(out=s1[off:off + sz, 0:ln - K],
                                  in_=rs[:sz, K:ln])

            for ch in range(nchunk - 1):
                sl = slice(ch * csz, (ch + 1) * csz)
                nc.vector.tensor_tensor_scan(
                    out=R[:, sl], data0=qc[:, sl], data1=d1[:, sl],
                    initial=0.0, op0=mult, op1=add,
                )
                nc.sync.dma_start(out=s0[:, sl], in_=R[:, sl])
            if cap:
                svscan(0, 0)
            sl = slice((nchunk - 1) * csz, FREE)
            nc.vector.tensor_tensor_scan(
                out=R[:, sl], data0=qc[:, sl], data1=d1[:, sl],
                initial=0.0, op0=mult, op1=add,
            )
            nc.sync.dma_start(out=s0[:, sl], in_=R[:, sl])
            if cap:
                off = surv_tiles[0][0]
                for ti, (sz, ln) in list(enumerate(surv_tiles))[1:]:
                    svscan(ti, off)
                    off += sz
    nc.compile()
    return nc


def get_nc(n_dd: int, surv_tiles: tuple):
    key = (n_dd, surv_tiles)
    if key not in _NC_CACHE:
        _NC_CACHE[key] = build_nc(n_dd, surv_tiles)
    return _NC_CACHE[key]


def _find_survivors(v: np.ndarray):
    """v: [1022] f32 (10*s).  Return j-indices with no exact-zero factor in
    rows m < K.  Factor zero <=> f32(v[j-1-m] + c_j) <= 0 (c = 1 - v), or,
    for the boundary rows (j <= m < K), c_j <= 0."""
    n = v.shape[0]
    c = (np.float32(1.0) - v).astype(np.float32)
    m = np.full(n, np.inf, dtype=np.float32)          # min of v over window
    if n > K:
        w = np.lib.stride_tricks.sliding_window_view(v, K).min(axis=1)
        m[K:] = w[:-1]                                # j >= K: v[j-K:j]
    run = np.minimum.accumulate(v)
    m[1:K] = run[:K - 1]                              # 0 < j < K: v[0:j]
    dead = (m + c).astype(np.float32) <= 0.0
    jk = np.arange(n) < K
    dead |= jk & (c <= 0.0)
    return np.nonzero(~dead)[0]


def prepare(score: np.ndarray, score_idx: np.ndarray):
    """Build (nc, in_maps, assemble) for the given inputs.  assemble(results)
    turns the per-core result dicts into the full output array."""
    score = np.asarray(score, dtype=np.float32)
    score_idx = np.asarray(score_idx)
    docs = score[score_idx]                  # [B, L]
    Bn, Ln = docs.shape
    assert Ln == L
    n_cores = 8
    dpc = Bn // n_cores                      # docs per core
    n_dd = dpc * 2
    assert n_dd == 8

    # per-core v arrays and survivor lists.  For each survivor also compute
    # its factor row and death row m_die (first exact-zero factor; reference
    # output is exactly 0 from m_die on, so the device scan stops there).
    vs = []            # vs[core][dd] = v (f32 [1022])
    survs = []         # survs[core] = list[(dd, j, m_die, factor_row)]
    for cid in range(n_cores):
        vcore = []
        for dl in range(dpc):
            s = docs[cid * dpc + dl, 1:-1].astype(np.float32)
            for t in range(2):
                sd = s if t == 0 else s[::-1]
                vcore.append((np.float32(10.0) * sd).astype(np.float32))
        slist = []
        for dd in range(n_dd):
            v = vcore[dd]
            for j in _find_survivors(v):
                j = int(j)
                cj = np.float32(1.0) - v[j]
                hat = np.zeros(ROWS, np.float32)
                if j > 0:
                    hat[:j] = v[j - 1::-1]
                fr = (hat + cj).astype(np.float32)
                z = np.nonzero(fr <= 0.0)[0]
                m_die = int(z[0]) if len(z) else ROWS
                slist.append((dd, j, m_die, fr))
        # longest-lived first, so later tiles get short scan lengths
        slist.sort(key=lambda e: -e[2])
        vs.append(vcore)
        survs.append(slist)

    # shared tile layout: sizes from the max core; per-tile scan length from
    # the max m_die in that slot range across ALL cores (rounded up)
    max_surv = max(len(s) for s in survs)
    tiles = []
    off = 0
    while off < max_surv:
        sz = min(P, max_surv - off)
        if sz < P:
            sz = max(32, -(-sz // 32) * 32)
        ln = K + 32
        for slist in survs:
            for e in slist[off:off + sz]:
                ln = max(ln, e[2])
        ln = min(ROWS, -(-ln // 32) * 32)
        tiles.append((sz, ln))
        off += sz
    surv_tiles = tuple(tiles)
    cap = sum(sz for sz, _ in surv_tiles)

    in_maps = []
    for cid in range(n_cores):
        arr = np.zeros((n_dd, ARRW), np.float32)
        for dd in range(n_dd):
            v = vs[cid][dd]
            arr[dd, K:K + N] = v
            arr[dd, COFF:COFF + N] = (np.float32(1.0) - v).astype(np.float32)
        im = {"arr": arr}
        if cap:
            scm = np.zeros((cap, L), np.float32)
            for slot, (dd, j, m_die, fr) in enumerate(survs[cid]):
                scm[slot, :ROWS] = fr
            im["sc"] = scm
        in_maps.append(im)

    nc = get_nc(n_dd, surv_tiles)

    def assemble(results):
        full = np.zeros((Bn, 2, ROWS, N), np.float32)
        for cid in range(n_cores):
            r = results[cid]
            # prefix: [128, FREE] -> [dd, slot, j', m] -> [dd, m, col]
            pref = np.asarray(r["s0"]).reshape(n_dd, 16, CJ, K)
            pref = pref.transpose(0, 3, 1, 2).reshape(n_dd, K, NCOL)[:, :, :N]
            for dd in range(n_dd):
                doc, t = cid * dpc + dd // 2, dd % 2
                full[doc, t, :K, :] = pref[dd]
            if cap:
                s1v = np.asarray(r["s1"])
                slot_ln = [ln for sz, ln in surv_tiles for _ in range(sz)]
                for slot, (dd, j, m_die, fr) in enumerate(survs[cid]):
                    doc, t = cid * dpc + dd // 2, dd % 2
                    ln = slot_ln[slot]
                    full[doc, t, K:ln, j] = s1v[slot, :ln - K]
        return full

    return nc, in_maps, assemble


def kernel(score: np.ndarray, score_idx: np.ndarray) -> np.ndarray:
    nc, in_maps, assemble = prepare(score, score_idx)
    res = bass_utils.run_bass_kernel_spmd(nc, in_maps, core_ids=list(range(8)))
    return assemble(res.results)
